# revision 1
# baseline (speedup 1.0000x reference)
"""Trainium2 Bass kernel for nn_AutoregulatedContinuum.

Data-parallel over 8 NeuronCores: x sharded along batch N; V_slow/gate/
regulator params replicated.  Per-core pipeline:

  phase A: v = x @ V_w.T  (fp32r matmuls, contraction on partitions via
           host-side transposes), streamed stats (sum x / sum x^2 /
           sum |v| on the scalar engine's accumulate path, the gate dot
           g = v . gate_w on the vector engine); v spilled to a DRAM
           scratch.
  allreduce: 4 partial sums over the 8 cores (tiny collective).
  regulator: stress/excitation/fatigue -> layernormed 2-layer MLP ->
           ctrl (computed redundantly on every core).
  phase B: out = sigmoid(g + gate_b) * strength * v.

W_fast is all zeros in this model family (the Hebbian branch contributes
exactly zero); if it is ever nonzero we fall back to a host reference.

DMA ring split: x-tiles + v spill/reload ride the scalar-engine HWDGE
ring, weights ride the sync-engine ring, small params ride gpsimd SWDGE
— so the first x tile is not queued behind 17 MB of weight loads.
"""

import numpy as np

DIM = 2048
N = 16384
NCORES = 8
RPC = N // NCORES            # rows per core
ITILES = RPC // 128          # 16 row-tiles per core
KTILES = DIM // 128          # 16 contraction tiles
JCH = 4                      # output column chunks of 512
WSLR = DIM // NCORES         # W_slow rows per core
WTILES = WSLR // 128         # 2
LN_EPS = 1e-5
NT = float(N) * float(DIM)

_CACHE = {}


def _build_program():
    import concourse.bacc as bacc
    import concourse.tile as tile
    import concourse.mybir as mybir
    from concourse import bass_isa

    F32 = mybir.dt.float32
    F32R = mybir.dt.float32r
    AX = mybir.AxisListType
    ALU = mybir.AluOpType
    ACT = mybir.ActivationFunctionType

    nc = bacc.Bacc("TRN2", target_bir_lowering=False, debug=False,
                   num_devices=NCORES)

    xt = nc.dram_tensor("xt", [DIM, RPC], F32R, kind="ExternalInput").ap()
    vwt = nc.dram_tensor("vwt", [DIM, DIM], F32R, kind="ExternalInput").ap()
    wsl = nc.dram_tensor("wsl", [WSLR, DIM], F32, kind="ExternalInput").ap()
    gwrep = nc.dram_tensor("gwrep", [128, DIM], F32, kind="ExternalInput").ap()
    gbrep = nc.dram_tensor("gbrep", [128, 1], F32, kind="ExternalInput").ap()
    r1wt = nc.dram_tensor("r1wt", [3, 16], F32, kind="ExternalInput").ap()
    r1b = nc.dram_tensor("r1b", [1, 16], F32, kind="ExternalInput").ap()
    lng = nc.dram_tensor("lng", [1, 16], F32, kind="ExternalInput").ap()
    lnb = nc.dram_tensor("lnb", [1, 16], F32, kind="ExternalInput").ap()
    r2wt = nc.dram_tensor("r2wt", [16, 3], F32, kind="ExternalInput").ap()
    r2b = nc.dram_tensor("r2b", [1, 3], F32, kind="ExternalInput").ap()
    out = nc.dram_tensor("out", [RPC, DIM], F32, kind="ExternalOutput").ap()

    xt3 = xt.rearrange("(t p) n -> p t n", p=128)     # [128, KTILES, RPC]

    with tile.TileContext(nc) as tc:
        with tc.tile_pool(name="const", bufs=1) as cst, \
             tc.tile_pool(name="dram", bufs=1, space="DRAM") as dram:

            # ---- small params (gpsimd SWDGE ring, off the critical path) ----
            gbr = cst.tile([128, 1], F32)
            nc.gpsimd.dma_start(gbr[:], gbrep[:, :])
            r1wt_s = cst.tile([3, 16], F32)
            nc.gpsimd.dma_start(r1wt_s[:], r1wt[:, :])
            r1b_s = cst.tile([1, 16], F32)
            nc.gpsimd.dma_start(r1b_s[:], r1b[:, :])
            lng_s = cst.tile([1, 16], F32)
            nc.gpsimd.dma_start(lng_s[:], lng[:, :])
            lnb_s = cst.tile([1, 16], F32)
            nc.gpsimd.dma_start(lnb_s[:], lnb[:, :])
            r2wt_s = cst.tile([16, 3], F32)
            nc.gpsimd.dma_start(r2wt_s[:], r2wt[:, :])
            r2b_s = cst.tile([1, 3], F32)
            nc.gpsimd.dma_start(r2b_s[:], r2b[:, :])
            ones1 = cst.tile([1, 128], F32)
            nc.vector.memset(ones1[:], 1.0)

            # ---- accumulators ----
            acc_x = cst.tile([128, ITILES], F32)
            acc_xx = cst.tile([128, ITILES], F32)
            acc_av = cst.tile([128, ITILES], F32)
            acc_w = cst.tile([128, WTILES], F32)
            g_mat = cst.tile([128, ITILES], F32)

            vscr = dram.tile([RPC, DIM], F32)

            # ---- W_slow Frobenius partial (gpsimd ring + ACT square-acc) ----
            with tc.tile_pool(name="wslp", bufs=2) as wslp:
                for t in range(WTILES):
                    wt = wslp.tile([128, DIM], F32, tag="wsl")
                    nc.gpsimd.dma_start(wt[:], wsl[t * 128:(t + 1) * 128, :])
                    wscr = wslp.tile([128, DIM], F32, tag="wscr")
                    nc.scalar.activation(wscr[:], wt[:], ACT.Square,
                                         accum_out=acc_w[:, t:t + 1])

            # ---- phase A: v matmul + stats + spill ----
            with tc.tile_pool(name="wpool", bufs=1) as wp, \
                 tc.tile_pool(name="xtp", bufs=2) as xtp, \
                 tc.tile_pool(name="vp", bufs=3) as vp, \
                 tc.tile_pool(name="scrp", bufs=1) as scrp, \
                 tc.tile_pool(name="scra", bufs=1) as scra, \
                 tc.tile_pool(name="psv", bufs=2, space="PSUM") as psv:
                # resident weights (sync ring: gwr first, then V_w.T)
                gwr = wp.tile([128, DIM], F32, tag="gwr")
                nc.sync.dma_start(gwr[:], gwrep[:, :])
                vwt_t = []
                for t in range(KTILES):
                    w = wp.tile([128, DIM], F32R, tag=f"vwt{t}")
                    nc.sync.dma_start(w[:], vwt[t * 128:(t + 1) * 128, :])
                    vwt_t.append(w)
                for i in range(ITILES):
                    xi = xtp.tile([128, DIM], F32R, tag="xi")
                    nc.scalar.dma_start(
                        xi[:].rearrange("p (t n) -> p t n", t=KTILES),
                        xt3[:, :, i * 128:(i + 1) * 128])
                    # batch stats on x via ACT accumulate
                    sa = scra.tile([128, DIM], F32, tag="sa")
                    nc.scalar.activation(sa[:], xi[:].bitcast(F32),
                                         ACT.Identity,
                                         accum_out=acc_x[:, i:i + 1])
                    sa2 = scra.tile([128, DIM], F32, tag="sa")
                    nc.scalar.activation(sa2[:], xi[:].bitcast(F32),
                                         ACT.Square,
                                         accum_out=acc_xx[:, i:i + 1])
                    # v row-tile
                    pv = psv.tile([128, DIM], F32, tag="pv")
                    for t in range(KTILES):
                        lhsT = xi[:, t * 128:(t + 1) * 128]
                        for j in range(JCH):
                            nc.tensor.matmul(
                                pv[:, j * 512:(j + 1) * 512], lhsT,
                                vwt_t[t][:, j * 512:(j + 1) * 512],
                                start=(t == 0), stop=(t == KTILES - 1))
                    vsb = vp.tile([128, DIM], F32, tag="vsb")
                    nc.vector.tensor_copy(vsb[:], pv[:])
                    # sum |v| via ACT accumulate
                    sa3 = scra.tile([128, DIM], F32, tag="sa")
                    nc.scalar.activation(sa3[:], vsb[:], ACT.Abs,
                                         accum_out=acc_av[:, i:i + 1])
                    # gate dot on DVE
                    scr2 = scrp.tile([128, DIM], F32, tag="scr")
                    nc.vector.tensor_mul(scr2[:], vsb[:], gwr[:])
                    nc.vector.tensor_reduce(
                        g_mat[:, i:i + 1], scr2[:], axis=AX.X, op=ALU.add)
                    nc.scalar.dma_start(vscr[i * 128:(i + 1) * 128, :], vsb[:])

            # ---- fold accumulators, cross-partition, allreduce ----
            sp = cst.tile([128, 4], F32)
            nc.vector.tensor_reduce(sp[:, 0:1], acc_x[:], axis=AX.X, op=ALU.add)
            nc.vector.tensor_reduce(sp[:, 1:2], acc_xx[:], axis=AX.X, op=ALU.add)
            nc.vector.tensor_reduce(sp[:, 2:3], acc_av[:], axis=AX.X, op=ALU.add)
            nc.vector.tensor_reduce(sp[:, 3:4], acc_w[:], axis=AX.X, op=ALU.add)
            onescol = cst.tile([128, 1], F32)
            nc.vector.memset(onescol[:], 1.0)
            arbuf = cst.tile([1, 8], F32)
            nc.vector.memset(arbuf[:], 0.0)
            with tc.tile_pool(name="psf", bufs=1, space="PSUM") as psf:
                pf = psf.tile([1, 4], F32, tag="pf")
                nc.tensor.matmul(pf[:], onescol[:, 0:1], sp[:])
                nc.scalar.copy(arbuf[0:1, 0:4], pf[0:1, :])
            tot = cst.tile([1, 8], F32)
            ccin = dram.tile([1, 8], F32)
            ccout = dram.tile([1, 8], F32)
            nc.sync.dma_start(ccin[:], arbuf[:])
            nc.gpsimd.collective_compute(
                "AllReduce", ALU.add,
                replica_groups=[list(range(NCORES))],
                ins=[ccin.opt()], outs=[ccout.opt()])
            nc.sync.dma_start(tot[:], ccout[:])

            # ---- regulator (redundant on every core) ----
            sig3 = cst.tile([1, 3], F32)
            mn = cst.tile([1, 1], F32)
            msq = cst.tile([1, 1], F32)
            ex2 = cst.tile([1, 1], F32)
            nc.scalar.mul(mn[:], tot[0:1, 0:1], 1.0 / NT)
            nc.vector.tensor_mul(msq[:], mn[:], mn[:])
            nc.scalar.mul(ex2[:], tot[0:1, 1:2], 1.0 / NT)
            nc.vector.tensor_sub(sig3[0:1, 0:1], ex2[:], msq[:])      # stress
            nc.scalar.mul(sig3[0:1, 1:2], tot[0:1, 2:3], 1.0 / NT)    # excitation
            nc.scalar.sqrt(sig3[0:1, 2:3], tot[0:1, 3:4])             # fatigue

            sigT = cst.tile([3, 1], F32)
            nc.gpsimd.dma_start(sigT[0:3, 0:1], sig3[0:1, 0:3])

            with tc.tile_pool(name="pss", bufs=1, space="PSUM") as pss:
                ph = pss.tile([1, 16], F32, tag="ph")
                nc.tensor.matmul(ph[:], sigT[0:3, 0:1], r1wt_s[0:3, :])
                h = cst.tile([1, 16], F32)
                nc.vector.tensor_add(h[:], ph[0:1, :], r1b_s[:])
                hm = cst.tile([1, 1], F32)
                nc.vector.tensor_reduce(hm[:], h[:], axis=AX.X, op=ALU.add)
                hm2 = cst.tile([1, 1], F32)
                nc.scalar.mul(hm2[:], hm[:], 1.0 / 16.0)
                hc = cst.tile([1, 16], F32)
                nc.vector.tensor_scalar_sub(hc[:], h[:], hm2[:])
                hc2 = cst.tile([1, 16], F32)
                hv = cst.tile([1, 1], F32)
                nc.vector.tensor_mul(hc2[:], hc[:], hc[:])
                nc.vector.tensor_reduce(hv[:], hc2[:], axis=AX.X, op=ALU.add)
                hv2 = cst.tile([1, 1], F32)
                nc.scalar.mul(hv2[:], hv[:], 1.0 / 16.0)
                hve = cst.tile([1, 1], F32)
                nc.vector.tensor_scalar_add(hve[:], hv2[:], LN_EPS)
                sd = cst.tile([1, 1], F32)
                nc.scalar.sqrt(sd[:], hve[:])
                rstd = cst.tile([1, 1], F32)
                nc.vector.reciprocal(rstd[:], sd[:])
                hn = cst.tile([1, 16], F32)
                nc.vector.tensor_scalar_mul(hn[:], hc[:], rstd[:])
                hg = cst.tile([1, 16], F32)
                nc.vector.tensor_mul(hg[:], hn[:], lng_s[:])
                hb = cst.tile([1, 16], F32)
                nc.vector.tensor_add(hb[:], hg[:], lnb_s[:])
                th = cst.tile([1, 16], F32)
                nc.scalar.activation(th[:], hb[:], ACT.Tanh)
                thT = cst.tile([16, 1], F32)
                nc.gpsimd.dma_start(thT[0:16, 0:1], th[0:1, 0:16])
                pc = pss.tile([1, 16], F32, tag="ph")
                nc.tensor.matmul(pc[0:1, 0:3], thT[0:16, 0:1], r2wt_s[0:16, :])
                cpre = cst.tile([1, 3], F32)
                nc.vector.tensor_add(cpre[:], pc[0:1, 0:3], r2b_s[:])
                ctrl = cst.tile([1, 3], F32)
                nc.scalar.activation(ctrl[:], cpre[:], ACT.Sigmoid)
                pb = pss.tile([128, 1], F32, tag="pb")
                nc.tensor.matmul(pb[:], ones1[0:1, 0:128], ctrl[0:1, 0:1])
                strb = cst.tile([128, 1], F32)
                nc.scalar.copy(strb[:], pb[:])

            # ---- gates ----
            glog = cst.tile([128, ITILES], F32)
            nc.vector.tensor_scalar_add(glog[:], g_mat[:], gbr[:, 0:1])
            gsig = cst.tile([128, ITILES], F32)
            nc.scalar.activation(gsig[:], glog[:], ACT.Sigmoid)
            gates = cst.tile([128, ITILES], F32)
            nc.vector.tensor_scalar_mul(gates[:], gsig[:], strb[:, 0:1])

            # ---- phase B: apply gates ----
            with tc.tile_pool(name="vbp", bufs=6) as vbp, \
                 tc.tile_pool(name="obp", bufs=3) as obp:
                for i in range(ITILES):
                    vi = vbp.tile([128, DIM], F32, tag="vi")
                    nc.scalar.dma_start(vi[:], vscr[i * 128:(i + 1) * 128, :])
                    ob = obp.tile([128, DIM], F32, tag="ob")
                    if i % 2 == 0:
                        nc.vector.tensor_scalar_mul(ob[:], vi[:],
                                                    gates[:, i:i + 1])
                    else:
                        nc.scalar.activation(ob[:], vi[:], ACT.Copy,
                                             scale=gates[:, i:i + 1])
                    nc.sync.dma_start(out[i * 128:(i + 1) * 128, :], ob[:])

    nc.compile()
    return nc


def _get_program():
    if "nc" not in _CACHE:
        _CACHE["nc"] = _build_program()
    return _CACHE["nc"]


def _host_reference(x, V_w, W_slow_w, gate_w, gate_b, r1_w, r1_b, ln_g,
                    ln_b, r2_w, r2_b, W_fast):
    """Numpy fallback for the (never-hit) W_fast != 0 case."""
    x = x.astype(np.float32)
    v = x @ V_w.T
    stress = x.var(dtype=np.float64).astype(np.float32)
    excitation = np.abs(v).mean(dtype=np.float64).astype(np.float32)
    fatigue = np.float32(np.linalg.norm(W_slow_w))
    s = np.array([[stress, excitation, fatigue]], np.float32)
    h = s @ r1_w.T + r1_b
    mu = h.mean(-1, keepdims=True)
    var = h.var(-1, keepdims=True)
    h = (h - mu) / np.sqrt(var + LN_EPS) * ln_g + ln_b
    h = np.tanh(h)
    ctrl = 1.0 / (1.0 + np.exp(-(h @ r2_w.T + r2_b)))
    ctrl = ctrl[0]
    gate = 1.0 / (1.0 + np.exp(-(v @ gate_w.T + gate_b))) * ctrl[0]
    n = np.float32(x.shape[0])
    y = x @ W_fast.T
    hebb = (y.T @ x) / n
    forget = np.mean(y * y, axis=0)[:, None] * W_fast
    Wf_new = W_fast + np.tanh(hebb - forget) * (ctrl[1] * np.float32(0.1))
    fast_out = x @ Wf_new.T
    return (gate * (v + fast_out * ctrl[2])).astype(np.float32)


def kernel(x, V_w, W_slow_w, gate_w, gate_b, r1_w, r1_b, ln_g, ln_b,
           r2_w, r2_b, W_fast):
    x = np.asarray(x, np.float32)
    V_w = np.asarray(V_w, np.float32)
    W_slow_w = np.asarray(W_slow_w, np.float32)
    gate_w = np.asarray(gate_w, np.float32)
    gate_b = np.asarray(gate_b, np.float32)
    W_fast = np.asarray(W_fast, np.float32)

    if np.any(W_fast):
        return _host_reference(x, V_w, W_slow_w, gate_w, gate_b,
                               np.asarray(r1_w, np.float32),
                               np.asarray(r1_b, np.float32),
                               np.asarray(ln_g, np.float32),
                               np.asarray(ln_b, np.float32),
                               np.asarray(r2_w, np.float32),
                               np.asarray(r2_b, np.float32), W_fast)

    in_maps = _prepare_inmaps(x, V_w, W_slow_w, gate_w, gate_b, r1_w, r1_b,
                              ln_g, ln_b, r2_w, r2_b)
    res = _run(in_maps)
    out = np.concatenate([res.results[c]["out"] for c in range(NCORES)],
                         axis=0)
    return out.astype(np.float32, copy=False)


def _run(in_maps, **kw):
    from concourse import bass_utils
    nc = _get_program()
    return bass_utils.run_bass_kernel_spmd(nc, in_maps,
                                           core_ids=list(range(NCORES)), **kw)


def _prepare_inmaps(x, V_w, W_slow_w, gate_w, gate_b, r1_w, r1_b, ln_g,
                    ln_b, r2_w, r2_b):
    vwt_h = np.ascontiguousarray(V_w.T)
    gwrep_h = np.ascontiguousarray(
        np.broadcast_to(gate_w.reshape(1, DIM), (128, DIM)))
    gbrep_h = np.full((128, 1), np.float32(gate_b.reshape(-1)[0]), np.float32)
    r1wt_h = np.ascontiguousarray(np.asarray(r1_w, np.float32).T)
    r1b_h = np.asarray(r1_b, np.float32).reshape(1, 16).copy()
    lng_h = np.asarray(ln_g, np.float32).reshape(1, 16).copy()
    lnb_h = np.asarray(ln_b, np.float32).reshape(1, 16).copy()
    r2wt_h = np.ascontiguousarray(np.asarray(r2_w, np.float32).T)
    r2b_h = np.asarray(r2_b, np.float32).reshape(1, 3).copy()

    in_maps = []
    for c in range(NCORES):
        in_maps.append({
            "xt": np.ascontiguousarray(x[c * RPC:(c + 1) * RPC, :].T),
            "vwt": vwt_h,
            "wsl": np.ascontiguousarray(W_slow_w[c * WSLR:(c + 1) * WSLR, :]),
            "gwrep": gwrep_h,
            "gbrep": gbrep_h,
            "r1wt": r1wt_h,
            "r1b": r1b_h,
            "lng": lng_h,
            "lnb": lnb_h,
            "r2wt": r2wt_h,
            "r2b": r2b_h,
        })

    return in_maps



# revision 2
# speedup vs baseline: 1.7985x; 1.7985x over previous
"""Trainium2 Bass kernel for nn_AutoregulatedContinuum.

Data-parallel over 8 NeuronCores: x sharded along batch N; V_slow/gate
params replicated.  W_fast is all zeros in this model family, so the
Hebbian branch contributes exactly zero and the computation reduces to

  v    = x @ V_w.T
  out  = sigmoid(v @ gate_w.T + gate_b) * strength * v

where strength = ctrl[0] of the regulator MLP driven by three global
scalars (var(x), mean|v|, ||W_slow||_F).

Single-phase device kernel (per core):
  - V_w.T resident in SBUF as bf16 (8 MB), x streamed as bf16 lhsT tiles
  - v row-tile accumulated in PSUM (fp32) via 16x4 bf16 matmuls
  - per tile, straight out of PSUM: sum|v| (ACT accumulate), gate logit
    (DVE mul+reduce vs replicated gate_w), sigmoid (ACT), and
    pre = sigmoid(logit) * v  (DVE scalar-mul, fused PSUM->SBUF copy)
  - pre spilled to HBM fp32; per-core sum|v| partials returned

Everything that needs the cross-core reduction (regulator MLP, the
global scalar `strength`) runs on the host during unsharding: stress and
fatigue come from the fp32 inputs directly, excitation from the summed
per-core partials, and the final output is strength * pre (a scalar
multiply applied while gathering shards).

If W_fast is ever nonzero we fall back to a host reference.
"""

import numpy as np

DIM = 2048
N = 16384
NCORES = 8
RPC = N // NCORES            # rows per core
ITILES = RPC // 128          # 16 row-tiles per core
KTILES = DIM // 128          # 16 contraction tiles
JCH = 4                      # output column chunks of 512
LN_EPS = 1e-5

_CACHE = {}


def _build_program():
    import concourse.bacc as bacc
    import concourse.tile as tile
    import concourse.mybir as mybir

    F32 = mybir.dt.float32
    BF16 = mybir.dt.bfloat16
    AX = mybir.AxisListType
    ALU = mybir.AluOpType
    ACT = mybir.ActivationFunctionType

    nc = bacc.Bacc("TRN2", target_bir_lowering=False, debug=False,
                   num_devices=NCORES)

    # xtl[i*128 + p, t*128 + r] = x_shard[i*128 + r, t*128 + p]
    # i.e. row-block i holds the 16 k-tile lhsT operands for that row tile,
    # contiguous so each per-tile DMA is one 512 KB linear transfer.
    xtl = nc.dram_tensor("xtl", [RPC, DIM], BF16, kind="ExternalInput").ap()
    vwt = nc.dram_tensor("vwt", [DIM, DIM], BF16, kind="ExternalInput").ap()
    gwrep = nc.dram_tensor("gwrep", [128, DIM], F32, kind="ExternalInput").ap()
    gbrep = nc.dram_tensor("gbrep", [128, 1], F32, kind="ExternalInput").ap()
    pre = nc.dram_tensor("pre", [RPC, DIM], F32, kind="ExternalOutput").ap()
    accav = nc.dram_tensor("accav", [128, ITILES], F32,
                           kind="ExternalOutput").ap()

    with tile.TileContext(nc) as tc:
        with tc.tile_pool(name="const", bufs=1) as cst:
            # small params ride the gpsimd SWDGE ring, off the HWDGE rings
            gwr = cst.tile([128, DIM], F32)
            nc.gpsimd.dma_start(gwr[:], gwrep[:, :])
            gbr = cst.tile([128, 1], F32)
            nc.gpsimd.dma_start(gbr[:], gbrep[:, :])
            acc_av = cst.tile([128, ITILES], F32)

            with tc.tile_pool(name="wpool", bufs=1) as wp, \
                 tc.tile_pool(name="xtp", bufs=3) as xtp, \
                 tc.tile_pool(name="scra", bufs=2) as scra, \
                 tc.tile_pool(name="scrp", bufs=2) as scrp, \
                 tc.tile_pool(name="pop", bufs=3) as pop, \
                 tc.tile_pool(name="gp", bufs=4) as gp, \
                 tc.tile_pool(name="psv", bufs=2, space="PSUM") as psv:
                # resident weights (sync ring)
                vwt_t = []
                for t in range(KTILES):
                    w = wp.tile([128, DIM], BF16, tag=f"vwt{t}")
                    nc.sync.dma_start(w[:], vwt[t * 128:(t + 1) * 128, :])
                    vwt_t.append(w)

                for i in range(ITILES):
                    xi = xtp.tile([128, DIM], BF16, tag="xi")
                    nc.scalar.dma_start(xi[:], xtl[i * 128:(i + 1) * 128, :])
                    # v row-tile: accumulate over k into fp32 PSUM
                    pv = psv.tile([128, DIM], F32, tag="pv")
                    for t in range(KTILES):
                        lhsT = xi[:, t * 128:(t + 1) * 128]
                        for j in range(JCH):
                            nc.tensor.matmul(
                                pv[:, j * 512:(j + 1) * 512], lhsT,
                                vwt_t[t][:, j * 512:(j + 1) * 512],
                                start=(t == 0), stop=(t == KTILES - 1))
                    # sum |v| partial via ACT accumulate (reads PSUM)
                    sa = scra.tile([128, DIM], F32, tag="sa")
                    nc.scalar.activation(sa[:], pv[:], ACT.Abs,
                                         accum_out=acc_av[:, i:i + 1])
                    # gate logit on DVE
                    scr = scrp.tile([128, DIM], F32, tag="scr")
                    nc.vector.tensor_mul(scr[:], pv[:], gwr[:])
                    gl = gp.tile([128, 1], F32, tag="gl")
                    nc.vector.tensor_reduce(gl[:], scr[:], axis=AX.X,
                                            op=ALU.add)
                    gs = gp.tile([128, 1], F32, tag="gs")
                    nc.scalar.activation(gs[:], gl[:], ACT.Sigmoid,
                                         bias=gbr[:, 0:1])
                    # pre = sigmoid(logit) * v, fused PSUM->SBUF evacuation
                    po = pop.tile([128, DIM], F32, tag="po")
                    nc.vector.tensor_scalar_mul(po[:], pv[:], gs[:, 0:1])
                    nc.sync.dma_start(pre[i * 128:(i + 1) * 128, :], po[:])

            nc.gpsimd.dma_start(accav[:, :], acc_av[:])

    nc.compile()
    return nc


def _get_program():
    if "nc" not in _CACHE:
        _CACHE["nc"] = _build_program()
    return _CACHE["nc"]


def _regulator_host(stress, excitation, fatigue, r1_w, r1_b, ln_g, ln_b,
                    r2_w, r2_b):
    sig = np.array([stress, excitation, fatigue], np.float64)
    h = sig @ np.asarray(r1_w, np.float64).T + np.asarray(r1_b, np.float64)
    mu = h.mean()
    var = h.var()
    h = (h - mu) / np.sqrt(var + LN_EPS) * np.asarray(ln_g, np.float64) \
        + np.asarray(ln_b, np.float64)
    h = np.tanh(h)
    z = h @ np.asarray(r2_w, np.float64).T + np.asarray(r2_b, np.float64)
    return 1.0 / (1.0 + np.exp(-z))


def _host_reference(x, V_w, W_slow_w, gate_w, gate_b, r1_w, r1_b, ln_g,
                    ln_b, r2_w, r2_b, W_fast):
    """Numpy fallback for the (never-hit) W_fast != 0 case."""
    x = x.astype(np.float32)
    v = x @ V_w.T
    stress = x.var(dtype=np.float64).astype(np.float32)
    excitation = np.abs(v).mean(dtype=np.float64).astype(np.float32)
    fatigue = np.float32(np.linalg.norm(W_slow_w))
    ctrl = _regulator_host(stress, excitation, fatigue, r1_w, r1_b, ln_g,
                           ln_b, r2_w, r2_b)
    gate = 1.0 / (1.0 + np.exp(-(v @ gate_w.T + gate_b))) * ctrl[0]
    n = np.float32(x.shape[0])
    y = x @ W_fast.T
    hebb = (y.T @ x) / n
    forget = np.mean(y * y, axis=0)[:, None] * W_fast
    Wf_new = W_fast + np.tanh(hebb - forget) * (ctrl[1] * np.float32(0.1))
    fast_out = x @ Wf_new.T
    return (gate * (v + fast_out * ctrl[2])).astype(np.float32)


def kernel(x, V_w, W_slow_w, gate_w, gate_b, r1_w, r1_b, ln_g, ln_b,
           r2_w, r2_b, W_fast):
    x = np.asarray(x, np.float32)
    V_w = np.asarray(V_w, np.float32)
    W_slow_w = np.asarray(W_slow_w, np.float32)
    gate_w = np.asarray(gate_w, np.float32)
    gate_b = np.asarray(gate_b, np.float32)
    W_fast = np.asarray(W_fast, np.float32)

    if np.any(W_fast):
        return _host_reference(x, V_w, W_slow_w, gate_w, gate_b,
                               np.asarray(r1_w, np.float32),
                               np.asarray(r1_b, np.float32),
                               np.asarray(ln_g, np.float32),
                               np.asarray(ln_b, np.float32),
                               np.asarray(r2_w, np.float32),
                               np.asarray(r2_b, np.float32), W_fast)

    in_maps = _prepare_inmaps(x, V_w, W_slow_w, gate_w, gate_b, r1_w, r1_b,
                              ln_g, ln_b, r2_w, r2_b)
    res = _run(in_maps)

    sumabs = float(sum(res.results[c]["accav"].astype(np.float64).sum()
                       for c in range(NCORES)))
    excitation = sumabs / (float(N) * float(DIM))
    stress = float(x.var(dtype=np.float64))
    fatigue = float(np.linalg.norm(W_slow_w.astype(np.float64)))
    ctrl = _regulator_host(stress, excitation, fatigue, r1_w, r1_b, ln_g,
                           ln_b, r2_w, r2_b)
    strength = np.float32(ctrl[0])

    out = np.concatenate([res.results[c]["pre"] for c in range(NCORES)],
                         axis=0)
    out = (out * strength).astype(np.float32, copy=False)
    return out


def _run(in_maps, **kw):
    from concourse import bass_utils
    nc = _get_program()
    return bass_utils.run_bass_kernel_spmd(nc, in_maps,
                                           core_ids=list(range(NCORES)), **kw)


def _prepare_inmaps(x, V_w, W_slow_w, gate_w, gate_b, r1_w, r1_b, ln_g,
                    ln_b, r2_w, r2_b):
    import ml_dtypes
    bf16 = ml_dtypes.bfloat16

    x = np.asarray(x, np.float32)
    vwt_h = np.ascontiguousarray(np.asarray(V_w, np.float32).T.astype(bf16))
    gwrep_h = np.ascontiguousarray(np.broadcast_to(
        np.asarray(gate_w, np.float32).reshape(1, DIM), (128, DIM)))
    gbrep_h = np.full((128, 1),
                      np.float32(np.asarray(gate_b).reshape(-1)[0]),
                      np.float32)

    in_maps = []
    for c in range(NCORES):
        xs = x[c * RPC:(c + 1) * RPC, :].reshape(ITILES, 128, KTILES, 128)
        # xtl[i*128 + p, t*128 + r] = x_shard[i*128 + r, t*128 + p]
        xtl = np.ascontiguousarray(
            xs.transpose(0, 3, 2, 1).reshape(RPC, DIM).astype(bf16))
        in_maps.append({
            "xtl": xtl,
            "vwt": vwt_h,
            "gwrep": gwrep_h,
            "gbrep": gbrep_h,
        })
    return in_maps
